# revision 19
# baseline (speedup 1.0000x reference)
"""Trainium2 Bass kernel for a single-head causal attention block.

Computes, per batch b:
    xn    = LayerNorm(x[b])           (non-affine, eps=1e-6)
    q,k,v = xn @ Wq, xn @ Wk, xn @ Wv
    s     = causal_mask(q @ k.T / sqrt(D))
    out   = softmax(s) @ v @ Wo + x[b]

Sharding (8 cores, SPMD single program):
  core c -> batch b = c//4, query stripe j = c%4 (queries {4k+j}).
  K/V ownership is chunked so the AllGather pipelines in two halves.

v2: the four projection matmul families (Q/K/V projections and the
output projection) run as fp8-e4m3 DoubleRow matmuls with a hi/lo
split: every operand a is represented as a_hi + a_lo (both e4m3,
prescaled so the lo part stays in e4m3's normal range), and products
keep the three terms ah*bh + ah*bl + al*bh.  A 256-deep contraction
pair costs 3 x 0.5 = 1.5 PE cycles/row instead of bf16's 2.0 (25%
fewer PE cycles) while carrying ~2x more mantissa than bf16, so
accuracy improves slightly.  Scales: xn is produced as 2*xn by folding
a factor 2 into the LayerNorm rsqrt; weights are prescaled by 8 on the
host; the resulting 16x on projection psums is folded into the staging
copies (1/16), so K/V/Q leave the projections at exactly the baseline
bf16 scale and the whole attention phase (scores, softmax, P@V) is
unchanged from the bf16 baseline.  The output projection takes
attention rows split hi/lo (unscaled, values ~3) against 8*Wo, with
the 1/8 folded into the fused residual add.

Attention structure (groups, AllGather pipelining, shared causal mask,
half-width diagonal tiles, ones-matmul softmax denominators) is
unchanged from the baseline; see git history for the long-form notes.
"""

import numpy as np
import ml_dtypes

import concourse.bacc as bacc
import concourse.tile as tile
from concourse import mybir
from concourse.bass_utils import run_bass_kernel_spmd

# Problem shape (hardcoded per harness contract)
B, S, H, D = 2, 4096, 2048, 2048
NCORES = 8
P = 128            # partitions
GQ = NCORES // B   # cores per batch = query stride
SQ = S // GQ       # queries per core
TQ = 256           # query group width
NGRP = SQ // TQ    # query groups per core (4)
HT = H // P        # h tiles (16)
DT = D // P        # d tiles (16)
NDIAG = TQ * GQ // P   # diagonal (mask) key tiles per query group (8)
MW = TQ + 32 * (NDIAG - 1)  # shared mask tile width (480)
CH = 512           # tokens per core per chunk
CHEL = CH * D      # elements per core-chunk contribution (per tensor)
PAD = 128          # dummy tail elements chaining AllGather #2 after #1

F32 = mybir.dt.float32
BF16 = mybir.dt.bfloat16
F8 = mybir.dt.float8e4
CDT = BF16
CDT_NP = ml_dtypes.bfloat16
F8_NP = ml_dtypes.float8_e4m3
DR = mybir.MatmulPerfMode.DoubleRow

EPS = 1e-6
NEG = -1e30
WPRE = 8.0         # host-side weight prescale
XPRE = 2.0         # xn prescale folded into the LN rsqrt
PSC = 1.0 / (WPRE * XPRE)   # projection psum descale


def build_nc(compile=True):
    nc = bacc.Bacc(num_devices=NCORES)

    # I/O.  xkv rows = [chunk1 tokens ; chunk2 tokens] for this core.
    xkv = nc.dram_tensor("xkv", [2 * CH, H], CDT, kind="ExternalInput")
    xq = nc.dram_tensor("xq", [SQ, H], CDT, kind="ExternalInput")
    # weights, hi/lo e4m3 pair layout [128, HT, N]: w[p, a, n] = W[a*128+p, n]
    wqh = nc.dram_tensor("wqh", [P, HT, D], F8, kind="ExternalInput")
    wql = nc.dram_tensor("wql", [P, HT, D], F8, kind="ExternalInput")
    wkh = nc.dram_tensor("wkh", [P, HT, D], F8, kind="ExternalInput")
    wkl = nc.dram_tensor("wkl", [P, HT, D], F8, kind="ExternalInput")
    wvh = nc.dram_tensor("wvh", [P, HT, D], F8, kind="ExternalInput")
    wvl = nc.dram_tensor("wvl", [P, HT, D], F8, kind="ExternalInput")
    woh = nc.dram_tensor("woh", [P, DT, H], F8, kind="ExternalInput")
    wol = nc.dram_tensor("wol", [P, DT, H], F8, kind="ExternalInput")
    masks = nc.dram_tensor("masks", [P, MW], F32, kind="ExternalInput")
    ident = nc.dram_tensor("ident", [P, P], CDT, kind="ExternalInput")
    out = nc.dram_tensor("out", [SQ, H], F32, kind="ExternalOutput")

    # DRAM scratch
    xn_d = nc.dram_tensor("xn_d", [2 * CH, H], CDT)
    xqn_d = nc.dram_tensor("xqn_d", [SQ, H], CDT)
    qt_d = nc.dram_tensor("qt_d", [2, D, SQ], F8)   # hi/lo Q^T
    # Per-chunk gather buffers: [0] = K^T as [DT,128,CH], [1] = V as [CH,D].
    agin = [nc.dram_tensor(f"agin{c}", [2, CHEL + PAD], CDT) for c in range(2)]
    agout = [
        nc.dram_tensor(f"agout{c}", [GQ, 2, CHEL + PAD], CDT) for c in range(2)
    ]
    CC_GROUPS = [list(range(g * GQ, (g + 1) * GQ)) for g in range(NCORES // GQ)]

    RSCALE = float(1.0 / np.sqrt(D))

    with (
        tile.TileContext(nc, pool_alloc_mode="queue") as tc,
        tc.tile_pool(name="consts", bufs=1) as consts,
        tc.tile_pool(name="wvo", bufs=1) as wvo_p,    # Wv then Wo slots
    ):
        ones = consts.tile([P, P], CDT)
        nc.vector.memset(ones, 1.0)
        eps_tile = consts.tile([P, 1], F32)
        nc.vector.memset(eps_tile, EPS / (XPRE * XPRE))
        masks_sb = consts.tile([P, MW], F32)
        ident_sb = consts.tile([P, P], CDT)
        zpad = consts.tile([P, 1], CDT)
        nc.vector.memset(zpad, 0.0)

        def load_w(pool, hi_d, lo_d, prefix, queues):
            """hi/lo packed weight pair -> two [P, HT, N] SBUF tiles."""
            nt = hi_d.shape[1]
            nn = hi_d.shape[2]
            th = pool.tile([P, nt, nn], F8, tag=f"{prefix}h")
            tl = pool.tile([P, nt, nn], F8, tag=f"{prefix}l")
            for i in range(4):
                sl = slice(i * (nt // 4), (i + 1) * (nt // 4))
                q = queues[i % len(queues)]
                q.dma_start(out=th[:, sl, :], in_=hi_d[:, sl, :])
                q.dma_start(out=tl[:, sl, :], in_=lo_d[:, sl, :])
            return th, tl

        def load_xt(pool, src_d, row0, xtmp_p):
            """[H, 512] block of (2*xn)^T, split hi/lo e4m3 packed."""
            th = pool.tile([P, HT, CH], F8, tag="xth")
            tl = pool.tile([P, HT, CH], F8, tag="xtl")
            for a in range(HT):
                xm = xtmp_p.tile([P, CH], CDT, tag="xm")
                nc.scalar.dma_start_transpose(
                    xm, src_d[row0 : row0 + CH, a * P : (a + 1) * P]
                )
                nc.scalar.activation(
                    out=th[:, a, :], in_=xm,
                    func=mybir.ActivationFunctionType.Copy,
                    bias=0.0, scale=1.0,
                )
                nc.vector.scalar_tensor_tensor(
                    out=tl[:, a, :], in0=xm, scalar=1.0, in1=th[:, a, :],
                    op0=mybir.AluOpType.mult,
                    op1=mybir.AluOpType.subtract,
                )
            return th, tl

        def mm3(ps, wh, wl, xh, xl, a2, wsl, xsl, first, last):
            """3-term hi/lo DoubleRow accumulation for contraction pair a2."""
            terms = [(wh, xh), (wh, xl), (wl, xh)]
            for i, (wt, xt) in enumerate(terms):
                nc.tensor.matmul(
                    ps, wt[:, 2 * a2 : 2 * a2 + 2, wsl],
                    xt[:, 2 * a2 : 2 * a2 + 2, xsl],
                    start=(first and i == 0), stop=(last and i == 2),
                    perf_mode=DR,
                )

        # ======== Phase 1: LayerNorm, projections, gathers ========
        with (
            tc.tile_pool(name="wkq", bufs=1) as wkq_p,   # Wk then Wq slots
            tc.tile_pool(name="xnT", bufs=2) as xnT_p,
            tc.tile_pool(name="pp1", bufs=6, space="PSUM") as pp1,
            tc.tile_pool(name="ppt", bufs=2, space="PSUM") as ppt,
            tc.tile_pool(name="xtmp", bufs=2) as xtmp_p,
        ):
            with (
                tc.tile_pool(name="xpool", bufs=2) as xpool,
                tc.tile_pool(name="xnpool", bufs=2) as xnpool,
                tc.tile_pool(name="stats", bufs=2) as stats_p,
                tc.tile_pool(name="small", bufs=4) as small_p,
                tc.tile_pool(name="stage1", bufs=5) as stage_p,
            ):
                def ln_rows(src, dst, t0, nt, ql=None, qs=None):
                    """LayerNorm token tiles [t0, t0+nt) of src -> dst,
                    scaled by XPRE (folded into the rsqrt)."""
                    ql = ql or nc.sync
                    qs = qs or nc.sync
                    xts = []
                    for t in range(t0, t0 + nt):
                        x_t = xpool.tile([P, H], CDT, tag="x")
                        ql.dma_start(
                            out=x_t, in_=src[t * P : (t + 1) * P, :]
                        )
                        xts.append(x_t)
                    for i, t in enumerate(range(t0, t0 + nt)):
                        x_t = xts[i]
                        stats = stats_p.tile([P, H // 512, 6], F32, tag="st")
                        for k in range(H // 512):
                            nc.vector.bn_stats(
                                out=stats[:, k, :],
                                in_=x_t[:, k * 512 : (k + 1) * 512],
                            )
                        mv = small_p.tile([P, 2], F32, tag="mv")
                        nc.vector.bn_aggr(out=mv, in_=stats)
                        sq = small_p.tile([P, 1], F32, tag="sq")
                        # sqrt((var+eps)/XPRE^2) => rs = XPRE/sqrt(var+eps)
                        nc.scalar.activation(
                            out=sq, in_=mv[:, 1:2],
                            func=mybir.ActivationFunctionType.Sqrt,
                            bias=eps_tile, scale=float(1.0 / (XPRE * XPRE)),
                        )
                        rs = small_p.tile([P, 1], F32, tag="rs")
                        nc.vector.reciprocal(out=rs, in_=sq)
                        xn_t = xnpool.tile([P, H], CDT, tag="xn")
                        nc.vector.tensor_scalar(
                            out=xn_t, in0=x_t, scalar1=mv[:, 0:1], scalar2=rs,
                            op0=mybir.AluOpType.subtract,
                            op1=mybir.AluOpType.mult,
                        )
                        qs.dma_start(
                            out=dst[t * P : (t + 1) * P, :], in_=xn_t
                        )

                def proj_kv(xth, xtl, ch, use_act):
                    """K^T and V projections for one chunk into agin[ch].
                    K is staged as an e4m3 hi/lo pair occupying the same
                    bytes the bf16 K used (bitcast view of agin)."""
                    ktv = agin[ch][0, :CHEL].bitcast(F8).rearrange(
                        "(l a p k) -> l a p k", a=DT, p=P, k=CH
                    )
                    vv = agin[ch][1, :CHEL].rearrange("(t d) -> t d", d=D)
                    # K^T: [128d, 512tok] tiles, hi/lo split (K-true scale)
                    for a in range(DT):
                        ps = pp1.tile([P, CH], F32, tag="ps")
                        for a2 in range(HT // 2):
                            mm3(ps, wk_h, wk_l, xth, xtl, a2,
                                slice(a * P, (a + 1) * P), slice(None),
                                a2 == 0, a2 == HT // 2 - 1)
                        sth = stage_p.tile([P, CH], F8, tag="sth")
                        nc.scalar.activation(
                            out=sth, in_=ps,
                            func=mybir.ActivationFunctionType.Copy,
                            bias=0.0, scale=float(PSC),
                        )
                        stl = stage_p.tile([P, CH], F8, tag="stl")
                        nc.vector.scalar_tensor_tensor(
                            out=stl, in0=ps, scalar=float(PSC), in1=sth,
                            op0=mybir.AluOpType.mult,
                            op1=mybir.AluOpType.subtract,
                        )
                        nc.sync.dma_start(out=ktv[0, a, :, :], in_=sth)
                        nc.sync.dma_start(out=ktv[1, a, :, :], in_=stl)
                    # V: [128tok, 512d] tiles
                    for tl in range(CH // P):
                        for dc in range(D // 512):
                            ps = pp1.tile([P, 512], F32, tag="ps")
                            for a2 in range(HT // 2):
                                mm3(ps, xth, xtl, wv_h, wv_l, a2,
                                    slice(tl * P, (tl + 1) * P),
                                    slice(dc * 512, (dc + 1) * 512),
                                    a2 == 0, a2 == HT // 2 - 1)
                            st = stage_p.tile([P, 512], CDT, tag="st")
                            cp_scale(st, ps, PSC, use_act)
                            nc.sync.dma_start(
                                out=vv[tl * P : (tl + 1) * P,
                                       dc * 512 : (dc + 1) * 512],
                                in_=st,
                            )

                def cp_scale(st, ps, scale, use_act):
                    if use_act:
                        nc.scalar.activation(
                            out=st, in_=ps,
                            func=mybir.ActivationFunctionType.Copy,
                            bias=0.0, scale=float(scale),
                        )
                    else:
                        nc.vector.tensor_scalar(
                            out=st, in0=ps, scalar1=float(scale), scalar2=None,
                            op0=mybir.AluOpType.mult,
                        )

                def proj_q(xth, xtl, qb):
                    """Q^T for query block qb, hi/lo e4m3 at Q-true scale
                    (the 1/sqrt(D) moves into the exp's activation scale)."""
                    for a in range(DT):
                        ps = pp1.tile([P, CH], F32, tag="ps")
                        for a2 in range(HT // 2):
                            mm3(ps, wq_h, wq_l, xth, xtl, a2,
                                slice(a * P, (a + 1) * P), slice(None),
                                a2 == 0, a2 == HT // 2 - 1)
                        sth = stage_p.tile([P, CH], F8, tag="sth")
                        nc.scalar.activation(
                            out=sth, in_=ps,
                            func=mybir.ActivationFunctionType.Copy,
                            bias=0.0, scale=float(PSC),
                        )
                        stl = stage_p.tile([P, CH], F8, tag="stl")
                        nc.vector.scalar_tensor_tensor(
                            out=stl, in0=ps, scalar=float(PSC), in1=sth,
                            op0=mybir.AluOpType.mult,
                            op1=mybir.AluOpType.subtract,
                        )
                        for hl, st in ((0, sth), (1, stl)):
                            nc.sync.dma_start(
                                out=qt_d[hl, a * P : (a + 1) * P,
                                         qb * CH : (qb + 1) * CH],
                                in_=st,
                            )

                def gather(ch):
                    nc.gpsimd.collective_compute(
                        "AllGather", mybir.AluOpType.bypass,
                        replica_groups=CC_GROUPS,
                        ins=[agin[ch][:, :]], outs=[agout[ch][:, :, :]],
                    )

                def tr_q(qb):
                    """(2*xqn)^T for query block qb via PE transposes,
                    split hi/lo e4m3 packed."""
                    th = xnT_p.tile([P, HT, CH], F8, tag="xth",
                                    name=f"xtqh{qb}")
                    tl2 = xnT_p.tile([P, HT, CH], F8, tag="xtl",
                                     name=f"xtql{qb}")
                    for t in range(CH // P):
                        for hh in range(2):
                            xm = xtmp_p.tile([P, H // 2], CDT, tag="xm")
                            nc.scalar.dma_start(
                                out=xm,
                                in_=xqn_d[qb * CH + t * P : qb * CH + (t + 1) * P,
                                          hh * (H // 2) : (hh + 1) * (H // 2)],
                            )
                            for a8 in range(HT // 2):
                                a = hh * (HT // 2) + a8
                                ps = ppt.tile([P, P], CDT, tag="pt")
                                nc.tensor.transpose(
                                    ps, xm[:, a8 * P : (a8 + 1) * P], ident_sb
                                )
                                sl = slice(t * P, (t + 1) * P)
                                nc.scalar.activation(
                                    out=th[:, a, sl], in_=ps,
                                    func=mybir.ActivationFunctionType.Copy,
                                    bias=0.0, scale=1.0,
                                )
                                nc.vector.scalar_tensor_tensor(
                                    out=tl2[:, a, sl], in0=ps, scalar=1.0,
                                    in1=th[:, a, sl],
                                    op0=mybir.AluOpType.mult,
                                    op1=mybir.AluOpType.subtract,
                                )
                    return th, tl2

                wk_h, wk_l = load_w(wkq_p, wkh, wkl, "k", [nc.gpsimd])
                nc.sync.dma_start(out=ident_sb, in_=ident[:, :])
                # zero the unwritten gather-pad tails
                for c, sub in ((0, 0), (0, 1), (1, 0), (1, 1)):
                    nc.sync.dma_start(
                        out=agin[c][sub, CHEL : CHEL + PAD].rearrange(
                            "(p f) -> p f", p=P
                        ),
                        in_=zpad,
                    )
                ln_rows(xkv, xn_d, 0, 4)
                ln_rows(xkv, xn_d, 4, 4)
                nc.sync.dma_start(out=masks_sb, in_=masks[:, :])
                wv_h, wv_l = load_w(wvo_p, wvh, wvl, "v", [nc.gpsimd])

                xt_c1 = load_xt(xnT_p, xn_d, 0, xtmp_p)
                proj_kv(*xt_c1, 0, True)
                ln_rows(xq, xqn_d, 0, 4, ql=nc.scalar, qs=nc.gpsimd)
                ln_rows(xq, xqn_d, 4, 4, ql=nc.scalar, qs=nc.gpsimd)
                gather(0)
                xt_c2 = load_xt(xnT_p, xn_d, CH, xtmp_p)
                wq_h, wq_l = load_w(wkq_p, wqh, wql, "k", [nc.scalar])
                proj_kv(*xt_c2, 1, False)
                xt_q1 = tr_q(0)
                proj_q(*xt_q1, 0)
                xt_q2 = tr_q(1)
                proj_q(*xt_q2, 1)
                gather(1)

        wo_h, wo_l = load_w(wvo_p, woh, wol, "v", [nc.sync])  # reuse Wv slots

        # ======== Phase 2: attention ========
        def kt_batch(ktc_p, kc):
            """Packed hi/lo kT tiles for key batch kc (keys [512kc,+512))."""
            c, r = divmod(kc, 4)
            kv = agout[c][r, 0, :CHEL].bitcast(F8).rearrange(
                "(l a p k) -> l a p k", a=DT, p=P, k=CH
            )
            q = nc.sync if kc % 2 == 0 else nc.scalar
            th = ktc_p.tile([P, DT, CH], F8, tag="kth")
            tl = ktc_p.tile([P, DT, CH], F8, tag="ktl")
            q.dma_start(out=th, in_=kv[0].rearrange("a p k -> p a k"))
            q.dma_start(out=tl, in_=kv[1].rearrange("a p k -> p a k"))
            return th, tl

        def vt_batch(vst_p, kc, d0):
            """V tiles [128tok, 4, 512] for key batch kc, d [d0,d0+512)."""
            c, r = divmod(kc, 4)
            vv = agout[c][r, 1, :CHEL].rearrange("(t d) -> t d", d=D)
            t = vst_p.tile([P, 4, 512], CDT, tag="vt")
            nc.scalar.dma_start(
                out=t,
                in_=vv[:, d0 : d0 + 512].rearrange("(tt p) d -> p tt d", p=P),
            )
            return t

        def m2_part(ktc_p, psc, g, qg, pT, sums, tk0, tk1):
            """Score pass for group g over key tiles [tk0, tk1)."""
            TK = NDIAG * (g + 1)
            gh = g % 2
            lag = []

            def flush_lag():
                for s_tk, s_qoff, s_nw in lag:
                    nc.tensor.matmul(
                        sums[:, s_qoff : s_qoff + s_nw], ones,
                        pT[:, s_tk, gh, s_qoff : s_qoff + s_nw],
                        start=(s_tk == 0), stop=(s_tk == TK - 1),
                        skip_group_check=True,
                    )
                lag.clear()

            for kc in range(tk0 // 4, tk1 // 4):
                kth, ktl = kt_batch(ktc_p, kc)
                for t4 in range(4):
                    tk = kc * 4 + t4
                    u = tk - (TK - NDIAG)
                    qoff = P if (u >= 4) else 0
                    ps = psc.tile([P, TQ], F32, tag="ps")
                    ksl = slice(t4 * P, (t4 + 1) * P)
                    for a2 in range(DT // 2):
                        asl = slice(2 * a2, 2 * a2 + 2)
                        terms = [
                            (kth, qg[0]), (kth, qg[1]), (ktl, qg[0])
                        ]
                        for i, (kt, qt) in enumerate(terms):
                            nc.tensor.matmul(
                                ps[:, qoff:], kt[:, asl, ksl],
                                qt[:, asl, qoff:],
                                start=(a2 == 0 and i == 0),
                                stop=(a2 == DT // 2 - 1 and i == 2),
                                perf_mode=DR,
                            )
                    if u >= 0:
                        s0 = 32 * (NDIAG - 1 - u)
                        nc.vector.tensor_add(
                            out=ps[:, qoff:], in0=ps[:, qoff:],
                            in1=masks_sb[:, s0 + qoff : s0 + TQ],
                        )
                        if qoff:
                            nc.vector.memset(pT[:, tk, gh, 0:qoff], 0.0)
                    nc.scalar.activation(
                        out=pT[:, tk, gh, qoff:], in_=ps[:, qoff:],
                        func=mybir.ActivationFunctionType.Exp,
                        bias=0.0, scale=float(RSCALE),
                    )
                    flush_lag()
                    lag.append((tk, qoff, TQ - qoff))
            flush_lag()

        def m3_sessions(vst_p, poa_p, pr, pT, tk0, tk1, sink):
            """P@V quarter-sessions for group pair pr over key tiles
            [tk0,tk1).  Tiles >= t_lo only feed the odd group."""
            t_lo = NDIAG * (2 * pr + 1)
            t_dg = NDIAG * (2 * pr + 1)  # odd group's diag base
            for qd in range(4):
                poas = [
                    poa_p.tile([P, 2, TQ], F32, tag=f"poa{d4}",
                               name=f"poa{pr}_{qd}_{d4}_{tk0}")
                    for d4 in range(4)
                ]
                for tk in range(tk0, tk1):
                    if tk % 4 == 0:
                        vt4 = vt_batch(vst_p, tk // 4, qd * 512)
                    vt = vt4[:, tk % 4, :]
                    for d4 in range(4):
                        if tk < t_lo:
                            o, r = poas[d4], pT[:, tk, :, :]
                        else:
                            qoff = P if (tk - t_dg >= 4) else 0
                            o = poas[d4][:, 1, qoff:]
                            r = pT[:, tk, 1, qoff:]
                        nc.tensor.matmul(
                            o, vt[:, d4 * P : (d4 + 1) * P], r,
                            start=(tk == tk0), stop=(tk == tk1 - 1),
                            skip_group_check=True,
                        )
                for d4 in range(4):
                    sink(qd * 4 + d4, poas[d4])

        def m4_group(res_p, ost_p, pfin, g, oaTh, oaTl):
            gh = g % 2
            for t2 in range(TQ // P):
                row0 = g * TQ + t2 * P
                res = res_p.tile([P, H], CDT, tag="res")
                nc.sync.dma_start(out=res, in_=xq[row0 : row0 + P, :])
                for hc in range(H // 512):
                    ps = pfin.tile([P, 512], F32, tag="ps")
                    hsl = slice(hc * 512, (hc + 1) * 512)
                    for a2 in range(DT // 2):
                        asl = slice(2 * a2, 2 * a2 + 2)
                        terms = [(oaTh, wo_h), (oaTh, wo_l), (oaTl, wo_h)]
                        for i, (oa, wt) in enumerate(terms):
                            nc.tensor.matmul(
                                ps, oa[:, asl, gh, t2 * P : (t2 + 1) * P],
                                wt[:, asl, hsl],
                                start=(a2 == 0 and i == 0),
                                stop=(a2 == DT // 2 - 1 and i == 2),
                                perf_mode=DR,
                            )
                    # ot = ps/WPRE + res  (fused)
                    ot = ost_p.tile([P, 512], F32, tag="ot")
                    nc.vector.scalar_tensor_tensor(
                        out=ot, in0=ps, scalar=float(1.0 / WPRE),
                        in1=res[:, hsl],
                        op0=mybir.AluOpType.mult,
                        op1=mybir.AluOpType.add,
                    )
                    nc.sync.dma_start(
                        out=out[row0 : row0 + P, hc * 512 : (hc + 1) * 512],
                        in_=ot,
                    )

        def load_qg(qg_p, g):
            th = qg_p.tile([P, DT, TQ], F8, tag="qgh", name=f"qgh{g}")
            tl = qg_p.tile([P, DT, TQ], F8, tag="qgl", name=f"qgl{g}")
            for hl, t in ((0, th), (1, tl)):
                nc.sync.dma_start(
                    out=t,
                    in_=qt_d[hl, :, g * TQ : (g + 1) * TQ].rearrange(
                        "(a p) t -> p a t", p=P
                    ),
                )
            return th, tl

        def split_oa(tmp_p, oaTh, oaTl, a, src):
            """src (f32) -> oaT hi/lo e4m3 at slot a."""
            nc.scalar.activation(
                out=oaTh[:, a, :, :], in_=src,
                func=mybir.ActivationFunctionType.Copy, bias=0.0, scale=1.0,
            )
            nc.vector.scalar_tensor_tensor(
                out=oaTl[:, a, :, :], in0=src, scalar=1.0,
                in1=oaTh[:, a, :, :],
                op0=mybir.AluOpType.mult,
                op1=mybir.AluOpType.subtract,
            )

        with (
            tc.tile_pool(name="ktc", bufs=2) as ktc_p,
            tc.tile_pool(name="vst", bufs=2) as vst_p,
            tc.tile_pool(name="qg", bufs=2) as qg_p,
            tc.tile_pool(name="rec", bufs=1) as rec_p,
            tc.tile_pool(name="res", bufs=1) as res_p,
            tc.tile_pool(name="ost", bufs=2) as ost_p,
            tc.tile_pool(name="oatmp", bufs=1) as oatmp_p,
            tc.tile_pool(name="spers", bufs=1, space="PSUM") as spers,
        ):
            sums23 = [
                spers.tile([P, TQ], F32, tag=f"sums{g}", name=f"sums{g}")
                for g in (2, 3)
            ]
            rec01 = rec_p.tile([P, 2, TQ], F32, tag="rec0")
            rec23 = rec_p.tile([P, 2, TQ], F32, tag="rec1")

            # ---- groups 0/1: fully AllGather-1 dependent ----
            with (
                tc.tile_pool(name="pt01", bufs=1) as pt01_p,
                tc.tile_pool(name="oa01", bufs=1) as oa01_p,
            ):
                pT01 = pt01_p.tile([P, 2 * NDIAG, 2, TQ], CDT, tag="pt")
                oaT01h = oa01_p.tile([P, DT, 2, TQ], F8, tag="oah")
                oaT01l = oa01_p.tile([P, DT, 2, TQ], F8, tag="oal")
                with (
                    tc.tile_pool(name="psc", bufs=3, space="PSUM") as psc,
                    tc.tile_pool(name="psm", bufs=1, space="PSUM") as psm,
                ):
                    for g in (0, 1):
                        qg = load_qg(qg_p, g)
                        sums = psm.tile([P, TQ], F32, tag="sums",
                                        name=f"sums{g}")
                        m2_part(ktc_p, psc, g, qg, pT01, sums,
                                0, NDIAG * (g + 1))
                        nc.vector.reciprocal(out=rec01[:, g, :], in_=sums)

                def sink01(a, poa):
                    t = oatmp_p.tile([P, 2, TQ], F32, tag="oat")
                    nc.vector.tensor_mul(out=t, in0=poa, in1=rec01)
                    split_oa(oatmp_p, oaT01h, oaT01l, a, t)

                with tc.tile_pool(name="poa", bufs=1, space="PSUM") as poa_p:
                    m3_sessions(vst_p, poa_p, 0, pT01, 0, 2 * NDIAG, sink01)
                with tc.tile_pool(name="pfin", bufs=2, space="PSUM") as pfin:
                    m4_group(res_p, ost_p, pfin, 0, oaT01h, oaT01l)
                    m4_group(res_p, ost_p, pfin, 1, oaT01h, oaT01l)

            # ---- groups 2/3: split across the AllGather-2 window ----
            with (
                tc.tile_pool(name="pt23", bufs=1) as pt23_p,
                tc.tile_pool(name="oa23", bufs=1) as oa23_p,
                tc.tile_pool(name="oal", bufs=1) as oal_p,
                tc.tile_pool(name="cmb", bufs=1) as cmb_p,
            ):
                pT23 = pt23_p.tile([P, 4 * NDIAG, 2, TQ], CDT, tag="pt")
                oaT23h = oa23_p.tile([P, DT, 2, TQ], F8, tag="oah")
                oaT23l = oa23_p.tile([P, DT, 2, TQ], F8, tag="oal")
                oal23 = oal_p.tile([P, DT, 2, TQ], CDT, tag="oalp")
                qg2 = load_qg(qg_p, 2)
                qg3 = load_qg(qg_p, 3)
                # window: chunk-1 scores for g2/g3
                with tc.tile_pool(name="psc2", bufs=3, space="PSUM") as psc2:
                    m2_part(ktc_p, psc2, 2, qg2, pT23, sums23[0], 0, 2 * NDIAG)
                    m2_part(ktc_p, psc2, 3, qg3, pT23, sums23[1], 0, 2 * NDIAG)

                # window: P@V partial over chunk-1 keys -> bf16
                def sink_lo(a, poa):
                    nc.vector.tensor_copy(oal23[:, a, :, :], poa)

                with tc.tile_pool(name="poa2", bufs=1, space="PSUM") as poa2_p:
                    m3_sessions(vst_p, poa2_p, 1, pT23, 0, 2 * NDIAG, sink_lo)

                # tail: AllGather-2 dependent.
                with tc.tile_pool(name="psc3", bufs=3, space="PSUM") as psc3:
                    m2_part(ktc_p, psc3, 2, qg2, pT23, sums23[0],
                            2 * NDIAG, 3 * NDIAG)
                    nc.vector.reciprocal(out=rec23[:, 0, :], in_=sums23[0])
                    m2_part(ktc_p, psc3, 3, qg3, pT23, sums23[1],
                            2 * NDIAG, 4 * NDIAG)
                    nc.vector.reciprocal(out=rec23[:, 1, :], in_=sums23[1])

                def sink_hi(a, poa):
                    t = cmb_p.tile([P, 2, TQ], F32, tag="cmb")
                    nc.vector.tensor_add(out=t, in0=poa, in1=oal23[:, a, :, :])
                    t2 = oatmp_p.tile([P, 2, TQ], F32, tag="oat")
                    nc.vector.tensor_mul(out=t2, in0=t, in1=rec23)
                    split_oa(oatmp_p, oaT23h, oaT23l, a, t2)

                with tc.tile_pool(name="poa3", bufs=1, space="PSUM") as poa3_p:
                    m3_sessions(vst_p, poa3_p, 1, pT23,
                                2 * NDIAG, 4 * NDIAG, sink_hi)
                with tc.tile_pool(name="pfin2", bufs=2, space="PSUM") as pfin2:
                    m4_group(res_p, ost_p, pfin2, 2, oaT23h, oaT23l)
                    m4_group(res_p, ost_p, pfin2, 3, oaT23h, oaT23l)

    if compile:
        nc.compile()
    return nc


def _make_masks(j):
    """Shared additive causal mask: m[r, s] = 0 iff r <= GQ*s + j - 896."""
    r = np.arange(P)[:, None]
    s = np.arange(MW)[None, :]
    return np.where(
        r <= GQ * s + j - GQ * 32 * (NDIAG - 1), 0.0, NEG
    ).astype(np.float32)


def _pack_w(w, nt):
    """w [K, N] f32 -> prescaled hi/lo e4m3 pair in [128, nt, N] layout."""
    wp = (w * WPRE).astype(np.float32)
    hi = wp.astype(F8_NP)
    lo = (wp - hi.astype(np.float32)).astype(F8_NP)
    n = w.shape[1]
    return (
        np.ascontiguousarray(hi.reshape(nt, P, n).transpose(1, 0, 2)),
        np.ascontiguousarray(lo.reshape(nt, P, n).transpose(1, 0, 2)),
    )


def _core_inputs(x, wpk, c):
    b, j = divmod(c, GQ)
    d = {
        "xkv": np.concatenate(
            [
                x[b, CH * j : CH * (j + 1), :],
                x[b, S // 2 + CH * j : S // 2 + CH * (j + 1), :],
            ]
        ).astype(CDT_NP),
        "xq": np.ascontiguousarray(x[b, j::GQ, :]).astype(CDT_NP),
        "masks": _make_masks(j),
        "ident": np.eye(P, dtype=CDT_NP),
    }
    d.update(wpk)
    return d


_NC_CACHE = None
_last_in_maps = None


def kernel(x, qkv, o_proj):
    global _NC_CACHE
    if _NC_CACHE is None:
        _NC_CACHE = build_nc()
    nc = _NC_CACHE

    x = np.ascontiguousarray(np.asarray(x, dtype=np.float32))
    qkv = np.asarray(qkv, dtype=np.float32)
    o_proj = np.asarray(o_proj, dtype=np.float32)
    wpk = {}
    wpk["wqh"], wpk["wql"] = _pack_w(np.ascontiguousarray(qkv[:, :D]), HT)
    wpk["wkh"], wpk["wkl"] = _pack_w(
        np.ascontiguousarray(qkv[:, D : 2 * D]), HT
    )
    wpk["wvh"], wpk["wvl"] = _pack_w(
        np.ascontiguousarray(qkv[:, 2 * D :]), HT
    )
    wpk["woh"], wpk["wol"] = _pack_w(o_proj, DT)

    in_maps = [_core_inputs(x, wpk, c) for c in range(NCORES)]

    global _last_in_maps
    _last_in_maps = in_maps
    res = run_bass_kernel_spmd(nc, in_maps, list(range(NCORES)))

    outp = np.empty((B, S, H), dtype=np.float32)
    for c in range(NCORES):
        b, j = divmod(c, GQ)
        outp[b, j::GQ, :] = res.results[c]["out"]
    return outp


# revision 22
# speedup vs baseline: 1.0094x; 1.0094x over previous
"""Trainium2 Bass kernel for a single-head causal attention block.

Computes, per batch b:
    xn    = LayerNorm(x[b])           (non-affine, eps=1e-6)
    q,k,v = xn @ Wq, xn @ Wk, xn @ Wv
    s     = causal_mask(q @ k.T / sqrt(D))
    out   = softmax(s) @ v @ Wo + x[b]

Sharding (8 cores, SPMD single program):
  core c -> batch b = c//4, query stripe j = c%4 (queries {4k+j}).
  K/V ownership is chunked so the AllGather pipelines in two halves.

v2: the four projection matmul families (Q/K/V projections and the
output projection) run as fp8-e4m3 DoubleRow matmuls with a hi/lo
split: every operand a is represented as a_hi + a_lo (both e4m3,
prescaled so the lo part stays in e4m3's normal range), and products
keep the three terms ah*bh + ah*bl + al*bh.  A 256-deep contraction
pair costs 3 x 0.5 = 1.5 PE cycles/row instead of bf16's 2.0 (25%
fewer PE cycles) while carrying ~2x more mantissa than bf16, so
accuracy improves slightly.  Scales: xn is produced as 2*xn by folding
a factor 2 into the LayerNorm rsqrt; weights are prescaled by 8 on the
host; the resulting 16x on projection psums is folded into the staging
copies (1/16), so K/V/Q leave the projections at exactly the baseline
bf16 scale and the whole attention phase (scores, softmax, P@V) is
unchanged from the bf16 baseline.  The output projection takes
attention rows split hi/lo (unscaled, values ~3) against 8*Wo, with
the 1/8 folded into the fused residual add.

Attention structure (groups, AllGather pipelining, shared causal mask,
half-width diagonal tiles, ones-matmul softmax denominators) is
unchanged from the baseline; see git history for the long-form notes.
"""

import numpy as np
import ml_dtypes

import concourse.bacc as bacc
import concourse.tile as tile
from concourse import mybir
from concourse.bass_utils import run_bass_kernel_spmd

# Problem shape (hardcoded per harness contract)
B, S, H, D = 2, 4096, 2048, 2048
NCORES = 8
P = 128            # partitions
GQ = NCORES // B   # cores per batch = query stride
SQ = S // GQ       # queries per core
TQ = 256           # query group width
NGRP = SQ // TQ    # query groups per core (4)
HT = H // P        # h tiles (16)
DT = D // P        # d tiles (16)
NDIAG = TQ * GQ // P   # diagonal (mask) key tiles per query group (8)
MW = TQ + 32 * (NDIAG - 1)  # shared mask tile width (480)
CH = 512           # tokens per core per chunk
CHEL = CH * D      # elements per core-chunk contribution (per tensor)
PAD = 128          # dummy tail elements chaining AllGather #2 after #1

F32 = mybir.dt.float32
BF16 = mybir.dt.bfloat16
F8 = mybir.dt.float8e4
CDT = BF16
CDT_NP = ml_dtypes.bfloat16
F8_NP = ml_dtypes.float8_e4m3
DR = mybir.MatmulPerfMode.DoubleRow

EPS = 1e-6
NEG = -1e30
WPRE = 8.0         # host-side weight prescale
XPRE = 2.0         # xn prescale folded into the LN rsqrt
PSC = 1.0 / (WPRE * XPRE)   # projection psum descale


def build_nc(compile=True):
    nc = bacc.Bacc(num_devices=NCORES)

    # I/O.  xkv rows = [chunk1 tokens ; chunk2 tokens] for this core.
    xkv = nc.dram_tensor("xkv", [2 * CH, H], CDT, kind="ExternalInput")
    xq = nc.dram_tensor("xq", [SQ, H], CDT, kind="ExternalInput")
    # weights, hi/lo e4m3 pair layout [128, HT, N]: w[p, a, n] = W[a*128+p, n]
    wqh = nc.dram_tensor("wqh", [P, HT, D], F8, kind="ExternalInput")
    wql = nc.dram_tensor("wql", [P, HT, D], F8, kind="ExternalInput")
    wkh = nc.dram_tensor("wkh", [P, HT, D], F8, kind="ExternalInput")
    wkl = nc.dram_tensor("wkl", [P, HT, D], F8, kind="ExternalInput")
    wvh = nc.dram_tensor("wvh", [P, HT, D], F8, kind="ExternalInput")
    wvl = nc.dram_tensor("wvl", [P, HT, D], F8, kind="ExternalInput")
    woh = nc.dram_tensor("woh", [P, DT, H], F8, kind="ExternalInput")
    wol = nc.dram_tensor("wol", [P, DT, H], F8, kind="ExternalInput")
    masks = nc.dram_tensor("masks", [P, MW], F32, kind="ExternalInput")
    ident = nc.dram_tensor("ident", [P, P], CDT, kind="ExternalInput")
    out = nc.dram_tensor("out", [SQ, H], F32, kind="ExternalOutput")

    # DRAM scratch
    xn_d = nc.dram_tensor("xn_d", [2 * CH, H], CDT)
    xqn_d = nc.dram_tensor("xqn_d", [SQ, H], CDT)
    qt_d = nc.dram_tensor("qt_d", [2, D, SQ], F8)   # hi/lo Q^T
    # Per-chunk gather buffers: [0] = K^T as [DT,128,CH], [1] = V as [CH,D].
    agin = [nc.dram_tensor(f"agin{c}", [2, CHEL + PAD], CDT) for c in range(2)]
    agout = [
        nc.dram_tensor(f"agout{c}", [GQ, 2, CHEL + PAD], CDT) for c in range(2)
    ]
    CC_GROUPS = [list(range(g * GQ, (g + 1) * GQ)) for g in range(NCORES // GQ)]

    RSCALE = float(1.0 / np.sqrt(D))

    with (
        tile.TileContext(nc, pool_alloc_mode="queue") as tc,
        tc.tile_pool(name="consts", bufs=1) as consts,
        tc.tile_pool(name="wvo", bufs=1) as wvo_p,    # Wv then Wo slots
    ):
        ones = consts.tile([P, P], CDT)
        nc.vector.memset(ones, 1.0)
        eps_tile = consts.tile([P, 1], F32)
        nc.vector.memset(eps_tile, EPS / (XPRE * XPRE))
        masks_sb = consts.tile([P, MW], F32)
        ident_sb = consts.tile([P, P], CDT)
        zpad = consts.tile([P, 1], CDT)
        nc.vector.memset(zpad, 0.0)

        def load_w(pool, hi_d, lo_d, prefix, queues):
            """hi/lo packed weight pair -> two [P, HT, N] SBUF tiles."""
            nt = hi_d.shape[1]
            nn = hi_d.shape[2]
            th = pool.tile([P, nt, nn], F8, tag=f"{prefix}h")
            tl = pool.tile([P, nt, nn], F8, tag=f"{prefix}l")
            for i in range(4):
                sl = slice(i * (nt // 4), (i + 1) * (nt // 4))
                q = queues[i % len(queues)]
                q.dma_start(out=th[:, sl, :], in_=hi_d[:, sl, :])
                q.dma_start(out=tl[:, sl, :], in_=lo_d[:, sl, :])
            return th, tl

        def load_xt(pool, src_d, row0, xtmp_p):
            """[H, 512] block of (2*xn)^T, split hi/lo e4m3 packed."""
            th = pool.tile([P, HT, CH], F8, tag="xth")
            tl = pool.tile([P, HT, CH], F8, tag="xtl")
            for a in range(HT):
                xm = xtmp_p.tile([P, CH], CDT, tag="xm")
                nc.scalar.dma_start_transpose(
                    xm, src_d[row0 : row0 + CH, a * P : (a + 1) * P]
                )
                nc.scalar.activation(
                    out=th[:, a, :], in_=xm,
                    func=mybir.ActivationFunctionType.Copy,
                    bias=0.0, scale=1.0,
                )
                nc.vector.scalar_tensor_tensor(
                    out=tl[:, a, :], in0=xm, scalar=1.0, in1=th[:, a, :],
                    op0=mybir.AluOpType.mult,
                    op1=mybir.AluOpType.subtract,
                )
            return th, tl

        def mm3(ps, wh, wl, xh, xl, a2, wsl, xsl, first, last):
            """3-term hi/lo DoubleRow accumulation for contraction pair a2."""
            terms = [(wh, xh), (wh, xl), (wl, xh)]
            for i, (wt, xt) in enumerate(terms):
                nc.tensor.matmul(
                    ps, wt[:, 2 * a2 : 2 * a2 + 2, wsl],
                    xt[:, 2 * a2 : 2 * a2 + 2, xsl],
                    start=(first and i == 0), stop=(last and i == 2),
                    perf_mode=DR,
                )

        # ======== Phase 1: LayerNorm, projections, gathers ========
        with (
            tc.tile_pool(name="wkq", bufs=1) as wkq_p,   # Wk then Wq slots
            tc.tile_pool(name="xnT", bufs=2) as xnT_p,
            tc.tile_pool(name="pp1", bufs=6, space="PSUM") as pp1,
            tc.tile_pool(name="ppt", bufs=2, space="PSUM") as ppt,
            tc.tile_pool(name="xtmp", bufs=2) as xtmp_p,
        ):
            with (
                tc.tile_pool(name="xpool", bufs=2) as xpool,
                tc.tile_pool(name="xnpool", bufs=2) as xnpool,
                tc.tile_pool(name="stats", bufs=2) as stats_p,
                tc.tile_pool(name="small", bufs=4) as small_p,
                tc.tile_pool(name="stage1", bufs=5) as stage_p,
            ):
                def ln_rows(src, dst, t0, nt, ql=None, qs=None):
                    """LayerNorm token tiles [t0, t0+nt) of src -> dst,
                    scaled by XPRE (folded into the rsqrt)."""
                    ql = ql or nc.sync
                    qs = qs or nc.sync
                    xts = []
                    for t in range(t0, t0 + nt):
                        x_t = xpool.tile([P, H], CDT, tag="x")
                        ql.dma_start(
                            out=x_t, in_=src[t * P : (t + 1) * P, :]
                        )
                        xts.append(x_t)
                    for i, t in enumerate(range(t0, t0 + nt)):
                        x_t = xts[i]
                        stats = stats_p.tile([P, H // 512, 6], F32, tag="st")
                        for k in range(H // 512):
                            nc.vector.bn_stats(
                                out=stats[:, k, :],
                                in_=x_t[:, k * 512 : (k + 1) * 512],
                            )
                        mv = small_p.tile([P, 2], F32, tag="mv")
                        nc.vector.bn_aggr(out=mv, in_=stats)
                        sq = small_p.tile([P, 1], F32, tag="sq")
                        # sqrt((var+eps)/XPRE^2) => rs = XPRE/sqrt(var+eps)
                        nc.scalar.activation(
                            out=sq, in_=mv[:, 1:2],
                            func=mybir.ActivationFunctionType.Sqrt,
                            bias=eps_tile, scale=float(1.0 / (XPRE * XPRE)),
                        )
                        rs = small_p.tile([P, 1], F32, tag="rs")
                        nc.vector.reciprocal(out=rs, in_=sq)
                        xn_t = xnpool.tile([P, H], CDT, tag="xn")
                        nc.vector.tensor_scalar(
                            out=xn_t, in0=x_t, scalar1=mv[:, 0:1], scalar2=rs,
                            op0=mybir.AluOpType.subtract,
                            op1=mybir.AluOpType.mult,
                        )
                        qs.dma_start(
                            out=dst[t * P : (t + 1) * P, :], in_=xn_t
                        )

                def proj_kv(xth, xtl, ch, use_act):
                    """K^T and V projections for one chunk into agin[ch].
                    K is staged as an e4m3 hi/lo pair occupying the same
                    bytes the bf16 K used (bitcast view of agin)."""
                    ktv = agin[ch][0, :CHEL].bitcast(F8).rearrange(
                        "(l a p k) -> l a p k", a=DT, p=P, k=CH
                    )
                    vv = agin[ch][1, :CHEL].rearrange("(t d) -> t d", d=D)
                    # K^T: [128d, 512tok] tiles, hi/lo split (K-true scale)
                    for a in range(DT):
                        ps = pp1.tile([P, CH], F32, tag="ps")
                        for a2 in range(HT // 2):
                            mm3(ps, wk_h, wk_l, xth, xtl, a2,
                                slice(a * P, (a + 1) * P), slice(None),
                                a2 == 0, a2 == HT // 2 - 1)
                        sth = stage_p.tile([P, CH], F8, tag="sth")
                        nc.scalar.activation(
                            out=sth, in_=ps,
                            func=mybir.ActivationFunctionType.Copy,
                            bias=0.0, scale=float(PSC),
                        )
                        stl = stage_p.tile([P, CH], F8, tag="stl")
                        nc.vector.scalar_tensor_tensor(
                            out=stl, in0=ps, scalar=float(PSC), in1=sth,
                            op0=mybir.AluOpType.mult,
                            op1=mybir.AluOpType.subtract,
                        )
                        nc.sync.dma_start(out=ktv[0, a, :, :], in_=sth)
                        nc.sync.dma_start(out=ktv[1, a, :, :], in_=stl)
                    # V: [128tok, 512d] tiles
                    for tl in range(CH // P):
                        for dc in range(D // 512):
                            ps = pp1.tile([P, 512], F32, tag="ps")
                            for a2 in range(HT // 2):
                                mm3(ps, xth, xtl, wv_h, wv_l, a2,
                                    slice(tl * P, (tl + 1) * P),
                                    slice(dc * 512, (dc + 1) * 512),
                                    a2 == 0, a2 == HT // 2 - 1)
                            st = stage_p.tile([P, 512], CDT, tag="st")
                            cp_scale(st, ps, PSC, use_act)
                            nc.sync.dma_start(
                                out=vv[tl * P : (tl + 1) * P,
                                       dc * 512 : (dc + 1) * 512],
                                in_=st,
                            )

                def cp_scale(st, ps, scale, use_act):
                    if use_act:
                        nc.scalar.activation(
                            out=st, in_=ps,
                            func=mybir.ActivationFunctionType.Copy,
                            bias=0.0, scale=float(scale),
                        )
                    else:
                        nc.vector.tensor_scalar(
                            out=st, in0=ps, scalar1=float(scale), scalar2=None,
                            op0=mybir.AluOpType.mult,
                        )

                def proj_q(xth, xtl, qb):
                    """Q^T for query block qb, hi/lo e4m3 at Q-true scale
                    (the 1/sqrt(D) moves into the exp's activation scale)."""
                    for a in range(DT):
                        ps = pp1.tile([P, CH], F32, tag="ps")
                        for a2 in range(HT // 2):
                            mm3(ps, wq_h, wq_l, xth, xtl, a2,
                                slice(a * P, (a + 1) * P), slice(None),
                                a2 == 0, a2 == HT // 2 - 1)
                        sth = stage_p.tile([P, CH], F8, tag="sth")
                        nc.scalar.activation(
                            out=sth, in_=ps,
                            func=mybir.ActivationFunctionType.Copy,
                            bias=0.0, scale=float(PSC),
                        )
                        stl = stage_p.tile([P, CH], F8, tag="stl")
                        nc.vector.scalar_tensor_tensor(
                            out=stl, in0=ps, scalar=float(PSC), in1=sth,
                            op0=mybir.AluOpType.mult,
                            op1=mybir.AluOpType.subtract,
                        )
                        for hl, st in ((0, sth), (1, stl)):
                            nc.sync.dma_start(
                                out=qt_d[hl, a * P : (a + 1) * P,
                                         qb * CH : (qb + 1) * CH],
                                in_=st,
                            )

                def gather(ch):
                    nc.gpsimd.collective_compute(
                        "AllGather", mybir.AluOpType.bypass,
                        replica_groups=CC_GROUPS,
                        ins=[agin[ch][:, :]], outs=[agout[ch][:, :, :]],
                    )

                def tr_q(qb):
                    """(2*xqn)^T for query block qb via PE transposes,
                    split hi/lo e4m3 packed."""
                    th = xnT_p.tile([P, HT, CH], F8, tag="xth",
                                    name=f"xtqh{qb}")
                    tl2 = xnT_p.tile([P, HT, CH], F8, tag="xtl",
                                     name=f"xtql{qb}")
                    for t in range(CH // P):
                        for hh in range(2):
                            xm = xtmp_p.tile([P, H // 2], CDT, tag="xm")
                            nc.scalar.dma_start(
                                out=xm,
                                in_=xqn_d[qb * CH + t * P : qb * CH + (t + 1) * P,
                                          hh * (H // 2) : (hh + 1) * (H // 2)],
                            )
                            for a8 in range(HT // 2):
                                a = hh * (HT // 2) + a8
                                ps = ppt.tile([P, P], CDT, tag="pt")
                                nc.tensor.transpose(
                                    ps, xm[:, a8 * P : (a8 + 1) * P], ident_sb
                                )
                                sl = slice(t * P, (t + 1) * P)
                                nc.scalar.activation(
                                    out=th[:, a, sl], in_=ps,
                                    func=mybir.ActivationFunctionType.Copy,
                                    bias=0.0, scale=1.0,
                                )
                                nc.vector.scalar_tensor_tensor(
                                    out=tl2[:, a, sl], in0=ps, scalar=1.0,
                                    in1=th[:, a, sl],
                                    op0=mybir.AluOpType.mult,
                                    op1=mybir.AluOpType.subtract,
                                )
                    return th, tl2

                wk_h, wk_l = load_w(wkq_p, wkh, wkl, "k", [nc.gpsimd])
                nc.sync.dma_start(out=ident_sb, in_=ident[:, :])
                # zero the unwritten gather-pad tails
                for c, sub in ((0, 0), (0, 1), (1, 0), (1, 1)):
                    nc.sync.dma_start(
                        out=agin[c][sub, CHEL : CHEL + PAD].rearrange(
                            "(p f) -> p f", p=P
                        ),
                        in_=zpad,
                    )
                ln_rows(xkv, xn_d, 0, 4)
                ln_rows(xkv, xn_d, 4, 4)
                nc.sync.dma_start(out=masks_sb, in_=masks[:, :])
                wv_h, wv_l = load_w(wvo_p, wvh, wvl, "v", [nc.gpsimd])

                xt_c1 = load_xt(xnT_p, xn_d, 0, xtmp_p)
                proj_kv(*xt_c1, 0, True)
                ln_rows(xq, xqn_d, 0, 4, ql=nc.scalar, qs=nc.gpsimd)
                ln_rows(xq, xqn_d, 4, 4, ql=nc.scalar, qs=nc.gpsimd)
                gather(0)
                xt_c2 = load_xt(xnT_p, xn_d, CH, xtmp_p)
                wq_h, wq_l = load_w(wkq_p, wqh, wql, "k", [nc.scalar])
                proj_kv(*xt_c2, 1, False)
                xt_q1 = tr_q(0)
                proj_q(*xt_q1, 0)
                xt_q2 = tr_q(1)
                proj_q(*xt_q2, 1)
                gather(1)

        wo_h, wo_l = load_w(wvo_p, woh, wol, "v", [nc.sync])  # reuse Wv slots

        # ======== Phase 2: attention ========
        def kt_batch(ktc_p, kc):
            """Packed hi/lo kT tiles for key batch kc (keys [512kc,+512))."""
            c, r = divmod(kc, 4)
            kv = agout[c][r, 0, :CHEL].bitcast(F8).rearrange(
                "(l a p k) -> l a p k", a=DT, p=P, k=CH
            )
            q = nc.sync if kc % 2 == 0 else nc.scalar
            th = ktc_p.tile([P, DT, CH], F8, tag="kth")
            tl = ktc_p.tile([P, DT, CH], F8, tag="ktl")
            q.dma_start(out=th, in_=kv[0].rearrange("a p k -> p a k"))
            q.dma_start(out=tl, in_=kv[1].rearrange("a p k -> p a k"))
            return th, tl

        def vt_batch(vst_p, kc, d0):
            """V tiles [128tok, 4, 512] for key batch kc, d [d0,d0+512)."""
            c, r = divmod(kc, 4)
            vv = agout[c][r, 1, :CHEL].rearrange("(t d) -> t d", d=D)
            t = vst_p.tile([P, 4, 512], CDT, tag="vt")
            nc.scalar.dma_start(
                out=t,
                in_=vv[:, d0 : d0 + 512].rearrange("(tt p) d -> p tt d", p=P),
            )
            return t

        def m2_part(ktc_p, psc, g, qg, pT, sums, tk0, tk1):
            """Score pass for group g over key tiles [tk0, tk1)."""
            TK = NDIAG * (g + 1)
            gh = g % 2
            lag = []

            def flush_lag():
                for s_tk, s_qoff, s_nw in lag:
                    nc.tensor.matmul(
                        sums[:, s_qoff : s_qoff + s_nw], ones,
                        pT[:, s_tk, gh, s_qoff : s_qoff + s_nw],
                        start=(s_tk == 0), stop=(s_tk == TK - 1),
                        skip_group_check=True,
                    )
                lag.clear()

            for kc in range(tk0 // 4, tk1 // 4):
                kth, ktl = kt_batch(ktc_p, kc)
                for t4 in range(4):
                    tk = kc * 4 + t4
                    u = tk - (TK - NDIAG)
                    qoff = P if (u >= 4) else 0
                    ps = psc.tile([P, TQ], F32, tag="ps")
                    ksl = slice(t4 * P, (t4 + 1) * P)
                    for a2 in range(DT // 2):
                        asl = slice(2 * a2, 2 * a2 + 2)
                        terms = [
                            (kth, qg[0]), (kth, qg[1]), (ktl, qg[0])
                        ]
                        for i, (kt, qt) in enumerate(terms):
                            nc.tensor.matmul(
                                ps[:, qoff:], kt[:, asl, ksl],
                                qt[:, asl, qoff:],
                                start=(a2 == 0 and i == 0),
                                stop=(a2 == DT // 2 - 1 and i == 2),
                                perf_mode=DR,
                            )
                    if u >= 0:
                        s0 = 32 * (NDIAG - 1 - u)
                        nc.vector.tensor_add(
                            out=ps[:, qoff:], in0=ps[:, qoff:],
                            in1=masks_sb[:, s0 + qoff : s0 + TQ],
                        )
                        if qoff:
                            nc.vector.memset(pT[:, tk, gh, 0:qoff], 0.0)
                    nc.scalar.activation(
                        out=pT[:, tk, gh, qoff:], in_=ps[:, qoff:],
                        func=mybir.ActivationFunctionType.Exp,
                        bias=0.0, scale=float(RSCALE),
                    )
                    flush_lag()
                    lag.append((tk, qoff, TQ - qoff))
            flush_lag()

        def m3_sessions(vst_p, poa_p, pr, pT, tk0, tk1, sink):
            """P@V quarter-sessions for group pair pr over key tiles
            [tk0,tk1).  Tiles >= t_lo only feed the odd group."""
            t_lo = NDIAG * (2 * pr + 1)
            t_dg = NDIAG * (2 * pr + 1)  # odd group's diag base
            for qd in range(4):
                poas = [
                    poa_p.tile([P, 2, TQ], F32, tag=f"poa{d4}",
                               name=f"poa{pr}_{qd}_{d4}_{tk0}")
                    for d4 in range(4)
                ]
                for tk in range(tk0, tk1):
                    if tk % 4 == 0:
                        vt4 = vt_batch(vst_p, tk // 4, qd * 512)
                    vt = vt4[:, tk % 4, :]
                    for d4 in range(4):
                        if tk < t_lo:
                            o, r = poas[d4], pT[:, tk, :, :]
                        else:
                            qoff = P if (tk - t_dg >= 4) else 0
                            o = poas[d4][:, 1, qoff:]
                            r = pT[:, tk, 1, qoff:]
                        nc.tensor.matmul(
                            o, vt[:, d4 * P : (d4 + 1) * P], r,
                            start=(tk == tk0), stop=(tk == tk1 - 1),
                            skip_group_check=True,
                        )
                for d4 in range(4):
                    sink(qd * 4 + d4, poas[d4])

        def m4_group(res_p, ost_p, pfin, g, oaTh, oaTl):
            gh = g % 2
            for t2 in range(TQ // P):
                row0 = g * TQ + t2 * P
                res = res_p.tile([P, H], CDT, tag="res")
                nc.sync.dma_start(out=res, in_=xq[row0 : row0 + P, :])
                for hc in range(H // 512):
                    ps = pfin.tile([P, 512], F32, tag="ps")
                    hsl = slice(hc * 512, (hc + 1) * 512)
                    for a2 in range(DT // 2):
                        asl = slice(2 * a2, 2 * a2 + 2)
                        terms = [(oaTh, wo_h), (oaTh, wo_l), (oaTl, wo_h)]
                        for i, (oa, wt) in enumerate(terms):
                            nc.tensor.matmul(
                                ps, oa[:, asl, gh, t2 * P : (t2 + 1) * P],
                                wt[:, asl, hsl],
                                start=(a2 == 0 and i == 0),
                                stop=(a2 == DT // 2 - 1 and i == 2),
                                perf_mode=DR,
                            )
                    # ot = ps/WPRE + res  (fused)
                    ot = ost_p.tile([P, 512], F32, tag="ot")
                    nc.vector.scalar_tensor_tensor(
                        out=ot, in0=ps, scalar=float(1.0 / WPRE),
                        in1=res[:, hsl],
                        op0=mybir.AluOpType.mult,
                        op1=mybir.AluOpType.add,
                    )
                    nc.sync.dma_start(
                        out=out[row0 : row0 + P, hc * 512 : (hc + 1) * 512],
                        in_=ot,
                    )

        def load_qg(qg_p, g):
            th = qg_p.tile([P, DT, TQ], F8, tag="qgh", name=f"qgh{g}")
            tl = qg_p.tile([P, DT, TQ], F8, tag="qgl", name=f"qgl{g}")
            for hl, t in ((0, th), (1, tl)):
                nc.sync.dma_start(
                    out=t,
                    in_=qt_d[hl, :, g * TQ : (g + 1) * TQ].rearrange(
                        "(a p) t -> p a t", p=P
                    ),
                )
            return th, tl

        def split_oa(tmp_p, oaTh, oaTl, a, src):
            """src (f32) -> oaT hi/lo e4m3 at slot a."""
            nc.scalar.activation(
                out=oaTh[:, a, :, :], in_=src,
                func=mybir.ActivationFunctionType.Copy, bias=0.0, scale=1.0,
            )
            nc.vector.scalar_tensor_tensor(
                out=oaTl[:, a, :, :], in0=src, scalar=1.0,
                in1=oaTh[:, a, :, :],
                op0=mybir.AluOpType.mult,
                op1=mybir.AluOpType.subtract,
            )

        with (
            tc.tile_pool(name="ktc", bufs=2) as ktc_p,
            tc.tile_pool(name="vst", bufs=3) as vst_p,
            tc.tile_pool(name="qg", bufs=2) as qg_p,
            tc.tile_pool(name="rec", bufs=1) as rec_p,
            tc.tile_pool(name="res", bufs=1) as res_p,
            tc.tile_pool(name="ost", bufs=2) as ost_p,
            tc.tile_pool(name="oatmp", bufs=1) as oatmp_p,
            tc.tile_pool(name="spers", bufs=1, space="PSUM") as spers,
        ):
            sums23 = [
                spers.tile([P, TQ], F32, tag=f"sums{g}", name=f"sums{g}")
                for g in (2, 3)
            ]
            rec01 = rec_p.tile([P, 2, TQ], F32, tag="rec0")
            rec23 = rec_p.tile([P, 2, TQ], F32, tag="rec1")

            # ---- groups 0/1: fully AllGather-1 dependent ----
            with (
                tc.tile_pool(name="pt01", bufs=1) as pt01_p,
                tc.tile_pool(name="oa01", bufs=1) as oa01_p,
            ):
                pT01 = pt01_p.tile([P, 2 * NDIAG, 2, TQ], CDT, tag="pt")
                oaT01h = oa01_p.tile([P, DT, 2, TQ], F8, tag="oah")
                oaT01l = oa01_p.tile([P, DT, 2, TQ], F8, tag="oal")
                with (
                    tc.tile_pool(name="psc", bufs=3, space="PSUM") as psc,
                    tc.tile_pool(name="psm", bufs=1, space="PSUM") as psm,
                ):
                    for g in (0, 1):
                        qg = load_qg(qg_p, g)
                        sums = psm.tile([P, TQ], F32, tag="sums",
                                        name=f"sums{g}")
                        m2_part(ktc_p, psc, g, qg, pT01, sums,
                                0, NDIAG * (g + 1))
                        nc.vector.reciprocal(out=rec01[:, g, :], in_=sums)

                def sink01(a, poa):
                    t = oatmp_p.tile([P, 2, TQ], F32, tag="oat")
                    nc.vector.tensor_mul(out=t, in0=poa, in1=rec01)
                    split_oa(oatmp_p, oaT01h, oaT01l, a, t)

                with tc.tile_pool(name="poa", bufs=1, space="PSUM") as poa_p:
                    m3_sessions(vst_p, poa_p, 0, pT01, 0, 2 * NDIAG, sink01)
                with tc.tile_pool(name="pfin", bufs=2, space="PSUM") as pfin:
                    m4_group(res_p, ost_p, pfin, 0, oaT01h, oaT01l)
                    m4_group(res_p, ost_p, pfin, 1, oaT01h, oaT01l)

            # ---- groups 2/3: split across the AllGather-2 window ----
            with (
                tc.tile_pool(name="pt23", bufs=1) as pt23_p,
                tc.tile_pool(name="oa23", bufs=1) as oa23_p,
                tc.tile_pool(name="oal", bufs=1) as oal_p,
                tc.tile_pool(name="cmb", bufs=1) as cmb_p,
            ):
                pT23 = pt23_p.tile([P, 4 * NDIAG, 2, TQ], CDT, tag="pt")
                oaT23h = oa23_p.tile([P, DT, 2, TQ], F8, tag="oah")
                oaT23l = oa23_p.tile([P, DT, 2, TQ], F8, tag="oal")
                oal23 = oal_p.tile([P, DT, 2, TQ], CDT, tag="oalp")
                qg2 = load_qg(qg_p, 2)
                qg3 = load_qg(qg_p, 3)
                # window: chunk-1 scores for g2/g3
                with tc.tile_pool(name="psc2", bufs=3, space="PSUM") as psc2:
                    m2_part(ktc_p, psc2, 2, qg2, pT23, sums23[0], 0, 2 * NDIAG)
                    m2_part(ktc_p, psc2, 3, qg3, pT23, sums23[1], 0, 2 * NDIAG)

                # window: P@V partial over chunk-1 keys -> bf16
                def sink_lo(a, poa):
                    nc.vector.tensor_copy(oal23[:, a, :, :], poa)

                with tc.tile_pool(name="poa2", bufs=1, space="PSUM") as poa2_p:
                    m3_sessions(vst_p, poa2_p, 1, pT23, 0, 2 * NDIAG, sink_lo)

                # tail: AllGather-2 dependent.
                tc.tile_set_cur_wait(0.75)
                with tc.tile_pool(name="psc3", bufs=3, space="PSUM") as psc3:
                    m2_part(ktc_p, psc3, 2, qg2, pT23, sums23[0],
                            2 * NDIAG, 3 * NDIAG)
                    nc.vector.reciprocal(out=rec23[:, 0, :], in_=sums23[0])
                    m2_part(ktc_p, psc3, 3, qg3, pT23, sums23[1],
                            2 * NDIAG, 4 * NDIAG)
                    nc.vector.reciprocal(out=rec23[:, 1, :], in_=sums23[1])

                def sink_hi(a, poa):
                    t = cmb_p.tile([P, 2, TQ], F32, tag="cmb")
                    nc.vector.tensor_add(out=t, in0=poa, in1=oal23[:, a, :, :])
                    t2 = oatmp_p.tile([P, 2, TQ], F32, tag="oat")
                    nc.vector.tensor_mul(out=t2, in0=t, in1=rec23)
                    split_oa(oatmp_p, oaT23h, oaT23l, a, t2)

                with tc.tile_pool(name="poa3", bufs=1, space="PSUM") as poa3_p:
                    m3_sessions(vst_p, poa3_p, 1, pT23,
                                2 * NDIAG, 4 * NDIAG, sink_hi)
                with tc.tile_pool(name="pfin2", bufs=2, space="PSUM") as pfin2:
                    m4_group(res_p, ost_p, pfin2, 2, oaT23h, oaT23l)
                    m4_group(res_p, ost_p, pfin2, 3, oaT23h, oaT23l)

    if compile:
        nc.compile()
    return nc


def _make_masks(j):
    """Shared additive causal mask: m[r, s] = 0 iff r <= GQ*s + j - 896."""
    r = np.arange(P)[:, None]
    s = np.arange(MW)[None, :]
    return np.where(
        r <= GQ * s + j - GQ * 32 * (NDIAG - 1), 0.0, NEG
    ).astype(np.float32)


def _pack_w(w, nt):
    """w [K, N] f32 -> prescaled hi/lo e4m3 pair in [128, nt, N] layout."""
    wp = (w * WPRE).astype(np.float32)
    hi = wp.astype(F8_NP)
    lo = (wp - hi.astype(np.float32)).astype(F8_NP)
    n = w.shape[1]
    return (
        np.ascontiguousarray(hi.reshape(nt, P, n).transpose(1, 0, 2)),
        np.ascontiguousarray(lo.reshape(nt, P, n).transpose(1, 0, 2)),
    )


def _core_inputs(x, wpk, c):
    b, j = divmod(c, GQ)
    d = {
        "xkv": np.concatenate(
            [
                x[b, CH * j : CH * (j + 1), :],
                x[b, S // 2 + CH * j : S // 2 + CH * (j + 1), :],
            ]
        ).astype(CDT_NP),
        "xq": np.ascontiguousarray(x[b, j::GQ, :]).astype(CDT_NP),
        "masks": _make_masks(j),
        "ident": np.eye(P, dtype=CDT_NP),
    }
    d.update(wpk)
    return d


_NC_CACHE = None
_last_in_maps = None


def kernel(x, qkv, o_proj):
    global _NC_CACHE
    if _NC_CACHE is None:
        _NC_CACHE = build_nc()
    nc = _NC_CACHE

    x = np.ascontiguousarray(np.asarray(x, dtype=np.float32))
    qkv = np.asarray(qkv, dtype=np.float32)
    o_proj = np.asarray(o_proj, dtype=np.float32)
    wpk = {}
    wpk["wqh"], wpk["wql"] = _pack_w(np.ascontiguousarray(qkv[:, :D]), HT)
    wpk["wkh"], wpk["wkl"] = _pack_w(
        np.ascontiguousarray(qkv[:, D : 2 * D]), HT
    )
    wpk["wvh"], wpk["wvl"] = _pack_w(
        np.ascontiguousarray(qkv[:, 2 * D :]), HT
    )
    wpk["woh"], wpk["wol"] = _pack_w(o_proj, DT)

    in_maps = [_core_inputs(x, wpk, c) for c in range(NCORES)]

    global _last_in_maps
    _last_in_maps = in_maps
    res = run_bass_kernel_spmd(nc, in_maps, list(range(NCORES)))

    outp = np.empty((B, S, H), dtype=np.float32)
    for c in range(NCORES):
        b, j = divmod(c, GQ)
        outp[b, j::GQ, :] = res.results[c]["out"]
    return outp
